# revision 15
# baseline (speedup 1.0000x reference)
"""LDA head (segment-reduce + Mahalanobis scores) on 8 Trainium2 NeuronCores.

Strategy (single SPMD NEFF on 8 cores, NO collectives):
  - Every core redundantly computes the full 1000-class stats from the full
    batch:  psS[65, 1000] = [z | 1]^T @ onehot  with 2x16 accumulating PE
    matmuls (fp16 operands; onehot built on DVE at 4x rate).
  - The (65, C) stats are transposed in 8 class chunks of 125 to a
    class-partition layout (fp16 staging, 1 cycle/row transposes), where all
    per-class postprocessing runs with per-partition scalars.
  - pooled variance uses  pooled*TSUM = sum_b z^2 - sum_c (n_c+2e)*mean_c^2,
    with the class-weighted reduction done as a matmul (lhsT=mean^2 chunks,
    rhs=(n+2e) column), so no per-class z^2 segment sums are needed.
  - V^T = [prec*mean | gamma] is built class-partitioned (prec broadcast via
    a rank-1 PE matmul + a middle-dim stride-0 tensor_tensor), transposed
    back in 8 chunks to V (65, 1000) fp16, and scores are one matmul per
    (j, h):  out = [z^T; 1]^T @ V  with  -0.5 * sum prec z^2  folded into
    the PSUM->SBUF copy as a per-partition bias.
  - Scores are batch-sharded: core k outputs rows [256k, 256k+256).
  - Only the natural_log activation table is used (Ln/Square/Copy/Identity)
    so there are no mid-kernel activation table loads.
  - Class half 0 is accumulated first so its transposes and postprocessing
    overlap the half-1 stats matmuls; PE warmup matmuls at t=0 ramp the
    p-state while the input DMAs land.

kernel(z, y) takes the full inputs and returns the full (2048, 1000) output.
"""

import sys
import numpy as np

if "/opt/trn_rl_repo" not in sys.path:
    sys.path.insert(0, "/opt/trn_rl_repo")

import concourse.bacc as bacc
import concourse.bass as bass
import concourse.mybir as mybir
from concourse import tile
from concourse.bass import broadcast_tensor_aps
from concourse.bass_utils import run_bass_kernel_spmd

B, C, D = 2048, 1000, 64
NCORES = 8
NT = B // 128               # 16 batch tiles
BL = B // NCORES            # 256 output rows per core
JT = BL // 128              # 2 local batch tiles
CH = C // 2                 # 500 column half
CL = 125                    # classes per transpose chunk
EPS_STATS = 1e-5
EPS_PREC = 1e-6
TSUM = float(np.float32(B) + np.float32(C * EPS_STATS))   # counts.sum()
FP = mybir.dt.float32
BF = mybir.dt.bfloat16
F16 = mybir.dt.float16
AF = mybir.ActivationFunctionType
ALU = mybir.AluOpType
WARMUP = 5


def build_program():
    nc = bacc.Bacc("TRN2", target_bir_lowering=False, debug=False,
                   num_devices=NCORES)

    zm_in = nc.dram_tensor("zm_in", [128, NT, D + 1], F16, kind="ExternalInput")
    czt_in = nc.dram_tensor("czt_in", [128, C + BL], F16, kind="ExternalInput")
    yc_in = nc.dram_tensor("yc_in", [128, NT], FP, kind="ExternalInput")
    out = nc.dram_tensor("out_loc", [BL, C], BF, kind="ExternalOutput")

    with tile.TileContext(nc) as tc:
        with tc.tile_pool(name="sb", bufs=1) as pool, \
             tc.tile_pool(name="ps", bufs=8, space="PSUM") as pp:

            # ---- warmup weights first so the PE ramp starts immediately
            wtile = pool.tile([128, CH], F16)
            nc.gpsimd.memset(wtile[:], 0.0)
            psW = pp.tile([D, CH], FP, tag="ps")
            for i in range(WARMUP):
                nc.tensor.matmul(psW[:], lhsT=wtile[:, 0:D], rhs=wtile[:],
                                 start=(i == 0), stop=(i == WARMUP - 1))

            # ---- input DMAs: combined cv+z^T on gpsimd, y + zm on sync
            czt = pool.tile([128, C + BL], F16)
            nc.gpsimd.dma_start(czt[:], czt_in[:, :])
            cv = czt[:, 0:C]
            zTq = czt[0:D + 1, C:C + BL]
            yc = pool.tile([128, NT], FP)
            nc.sync.dma_start(yc[:], yc_in[:, :])
            zm = pool.tile([128, NT, D + 1], F16)
            nc.sync.dma_start(zm[:], zm_in[:, :, :])

            # ---- on-device constants
            fiota = pool.tile([128, 128], FP)
            nc.gpsimd.iota(fiota[:], [[1, 128]], base=0, channel_multiplier=0,
                           allow_small_or_imprecise_dtypes=True)
            piota = pool.tile([128, 1], FP)
            nc.gpsimd.iota(piota[:], [[1, 1]], base=0, channel_multiplier=1,
                           allow_small_or_imprecise_dtypes=True)
            ones16 = pool.tile([128, 1], F16)
            nc.gpsimd.memset(ones16[:], 1.0)
            ones32 = pool.tile([1, 1], FP)
            nc.gpsimd.memset(ones32[:], 1.0)
            onesr = pool.tile([1, 128], FP)
            nc.gpsimd.memset(onesr[:], 1.0)
            idn = pool.tile([128, 128], FP)
            nc.vector.tensor_scalar(idn[:], fiota[:], piota[:, 0:1], None,
                                    ALU.is_equal)
            idn16 = pool.tile([128, 128], F16)
            nc.vector.tensor_scalar(idn16[:], fiota[:], piota[:, 0:1], None,
                                    ALU.is_equal)

            # force the (single) activation table load while ACT is idle
            dscr = pool.tile([1, 1], FP)
            nc.scalar.activation(dscr[:], ones32[:], AF.Ln)

            # ---- z^2 (full batch, for pooled term1) + local z^2 (64, 256)
            z2 = pool.tile([128, NT, D], F16)
            nc.scalar.activation(z2[:], zm[:, :, 0:D], AF.Square)
            z2T = pool.tile([D, BL], F16)
            nc.scalar.activation(z2T[:], zTq[0:D, :], AF.Square)

            # ---- one-hot build: 16 x (128, 1000) fp16 at DVE 4x rate
            oh = pool.tile([128, NT, C], F16)
            for t in range(NT):
                nc.vector.tensor_scalar(oh[:, t, :], cv, yc[:, t:t + 1],
                                        None, ALU.is_equal)

            # fold z^2 over tiles (DVE tree, fp16 2x) -> zf (128, 64)
            f8 = pool.tile([128, 8, D], F16)
            nc.vector.tensor_tensor(f8[:], z2[:, 0:8, :], z2[:, 8:16, :],
                                    ALU.add)
            f4 = pool.tile([128, 4, D], F16)
            nc.vector.tensor_tensor(f4[:], f8[:, 0:4, :], f8[:, 4:8, :],
                                    ALU.add)
            f2 = pool.tile([128, 2, D], F16)
            nc.vector.tensor_tensor(f2[:], f4[:, 0:2, :], f4[:, 2:4, :],
                                    ALU.add)
            zf = pool.tile([128, D], F16)
            nc.vector.tensor_tensor(zf[:], f2[:, 0, :], f2[:, 1, :], ALU.add)

            # shared postproc tiles
            psScp = pool.tile([D + 1, C], F16)
            meanT = pool.tile([CL, 8, D], FP)
            msq = pool.tile([CL, 8, D], F16)
            g0t = pool.tile([CL, 8, 1], FP)
            gs = pool.tile([CL, 8, 1], FP)
            VT = pool.tile([CL, 8, D + 1], F16)
            junk2 = pool.tile([CL, 8, D], FP)
            rcp = [None, None]
            c2 = [None, None]

            def postproc_half(h, psTh):
                """counts -> rcp / (n+2e) / log-prior; mean and mean^2."""
                cnte = pool.tile([CL, 4, 1], FP, tag=f"cnte{h}", name=f"cnte{h}")
                nc.vector.tensor_scalar_add(cnte[:], psTh[:, :, D:D + 1],
                                            EPS_STATS)
                rcp[h] = pool.tile([CL, 4, 1], FP, tag=f"rcp{h}", name=f"rcp{h}")
                nc.vector.reciprocal(rcp[h][:], cnte[:])
                nc.scalar.activation(g0t[:, 4 * h:4 * h + 4, :], cnte[:],
                                     AF.Ln, scale=1.0 / TSUM)
                c2[h] = pool.tile([CL, 4, 1], F16, tag=f"c2{h}", name=f"c2{h}")
                nc.vector.tensor_scalar_add(c2[h][:], cnte[:], EPS_STATS)
                for i in range(4):
                    nc.vector.tensor_scalar(meanT[:, 4 * h + i, :],
                                            psTh[:, i, 0:D], rcp[h][:, i, :],
                                            None, ALU.mult)
                nc.vector.tensor_tensor(msq[:, 4 * h:4 * h + 4, :],
                                        meanT[:, 4 * h:4 * h + 4, :],
                                        meanT[:, 4 * h:4 * h + 4, :],
                                        ALU.mult)

            # ---- stats half 0: psS0 = [z|1]^T @ onehot[:, 0:500]
            psS0 = pp.tile([D + 1, CH], FP, tag="ps")
            for t in range(NT):
                nc.tensor.matmul(psS0[:], lhsT=zm[:, t, :],
                                 rhs=oh[:, t, 0:CH],
                                 start=(t == 0), stop=(t == NT - 1))
            nc.scalar.copy(psScp[:, 0:CH], psS0[:])
            psT0 = pp.tile([CL, 4, D + 2], F16, tag="ps")
            for i in range(4):
                nc.tensor.matmul(psT0[:, i, 0:D + 1],
                                 lhsT=psScp[:, i * CL:(i + 1) * CL],
                                 rhs=idn16[0:D + 1, 0:D + 1],
                                 is_transpose=True, skip_group_check=True)
            postproc_half(0, psT0)          # overlaps the half-1 stats below

            # ---- stats half 1
            psS1 = pp.tile([D + 1, CH], FP, tag="ps")
            for t in range(NT):
                nc.tensor.matmul(psS1[:], lhsT=zm[:, t, :],
                                 rhs=oh[:, t, CH:C],
                                 start=(t == 0), stop=(t == NT - 1))
            psZ = pp.tile([D, 1], FP, tag="ps")
            nc.tensor.matmul(psZ[:], lhsT=zf[:], rhs=ones16[:],
                             start=True, stop=True)
            nc.scalar.copy(psScp[:, CH:C], psS1[:])
            psT1 = pp.tile([CL, 4, D + 2], F16, tag="ps")
            for i in range(4):
                nc.tensor.matmul(psT1[:, i, 0:D + 1],
                                 lhsT=psScp[:, CH + i * CL:CH + (i + 1) * CL],
                                 rhs=idn16[0:D + 1, 0:D + 1],
                                 is_transpose=True, skip_group_check=True)
            postproc_half(1, psT1)
            psT = [psT0, psT1]

            # ---- pooled variance + precision (global over all classes)
            # sum_c (n_c+2e) mean_c^2 as 8 matmuls: lhsT=msq, rhs=(n+2e)
            psPP = pp.tile([D, 8], FP, tag="ps")
            for g in range(8):
                nc.tensor.matmul(psPP[:, g:g + 1], lhsT=msq[:, g, :],
                                 rhs=c2[g // 4][:, g % 4, :],
                                 start=True, stop=True, skip_group_check=True)
            asum = pool.tile([D, 1], FP)
            nc.vector.reduce_sum(asum[:], psPP[:], axis=mybir.AxisListType.X)
            pooled = pool.tile([D, 1], FP)
            nc.vector.tensor_tensor(pooled[:], psZ[:], asum[:], ALU.subtract)
            nc.vector.tensor_scalar(pooled[:], pooled[:], 1.0 / TSUM,
                                    EPS_STATS, ALU.mult, ALU.add)
            pmax = pool.tile([D, 1], FP)
            nc.vector.tensor_scalar_max(pmax[:], pooled[:], EPS_PREC)
            prec = pool.tile([D, 1], FP)
            nc.vector.reciprocal(prec[:], pmax[:])
            nhp16 = pool.tile([D, 1], F16)
            nc.vector.tensor_scalar(nhp16[:], prec[:], -0.5, None, ALU.mult)

            # prec broadcast tile (128, 64) via transpose + rank-1 matmul
            psPR = pp.tile([1, D], FP, tag="ps")
            nc.tensor.matmul(psPR[:], lhsT=prec[:], rhs=idn[0:D, 0:D],
                             is_transpose=True, skip_group_check=True)
            prow = pool.tile([1, D], FP)
            nc.scalar.copy(prow[:], psPR[:])
            psPB = pp.tile([128, D], FP, tag="ps")
            nc.tensor.matmul(psPB[:], lhsT=onesr[:], rhs=prow[:],
                             start=True, stop=True)
            precbc = pool.tile([128, 1, D], FP)
            nc.vector.tensor_copy(precbc[:, 0, :], psPB[:])

            # local quadratic row -> per-row bias qsb (128, JT)
            psQ = pp.tile([1, BL], FP, tag="ps")
            nc.tensor.matmul(psQ[:], lhsT=nhp16[:], rhs=z2T[:],
                             start=True, stop=True)
            qrowS = pool.tile([1, BL], F16)
            nc.scalar.copy(qrowS[:], psQ[:])
            psQT = pp.tile([128, JT], FP, tag="ps")
            for j in range(JT):
                nc.tensor.matmul(psQT[:, j:j + 1],
                                 lhsT=qrowS[:, j * 128:(j + 1) * 128],
                                 rhs=ones16[0:1, :],
                                 start=True, stop=True,
                                 skip_group_check=True)
            qsb = pool.tile([128, JT], FP)
            nc.scalar.copy(qsb[:], psQT[:])

            # ---- V^T build + transpose back, half by half
            V = pool.tile([D + 1, C], F16)
            psV = [None, None]
            for h in range(2):
                hs = slice(4 * h, 4 * h + 4)
                mh = meanT[:, hs, :]
                pb = precbc[0:CL, 0:1, :]
                b0, b1 = broadcast_tensor_aps(mh, pb)
                nc.vector.tensor_tensor(VT[:, hs, 0:D], b0, b1, ALU.mult)
                m0, m1 = broadcast_tensor_aps(msq[:, hs, :], pb)
                nc.vector.tensor_tensor(junk2[:, hs, :], m0, m1, ALU.mult)
                nc.vector.reduce_sum(gs[:, hs, :], junk2[:, hs, :],
                                     axis=mybir.AxisListType.X)
                nc.vector.tensor_scalar(gs[:, hs, :], gs[:, hs, :], -0.5,
                                        None, ALU.mult)
                nc.vector.tensor_tensor(VT[:, hs, D:D + 1],
                                        g0t[:, hs, :], gs[:, hs, :],
                                        ALU.add)
                psV[h] = pp.tile([D + 1, 4, CL + 1], F16, tag="ps",
                                 name=f"psV{h}")
                for i in range(4):
                    nc.tensor.matmul(psV[h][:, i, 0:CL],
                                     lhsT=VT[:, 4 * h + i, :],
                                     rhs=idn16[0:CL, 0:CL],
                                     is_transpose=True, skip_group_check=True)
                for i in range(4):
                    dst = V[:, h * CH + i * CL:h * CH + (i + 1) * CL]
                    if i % 2 == 0:
                        nc.scalar.copy(dst, psV[h][:, i, 0:CL])
                    else:
                        nc.vector.tensor_copy(dst, psV[h][:, i, 0:CL])

            # ---- scores + output
            out_eng = [nc.sync, nc.gpsimd]
            for h in range(2):
                for j in range(JT):
                    psO = pp.tile([128, CH], FP, tag="ps", name=f"psO{j}{h}")
                    nc.tensor.matmul(psO[:],
                                     lhsT=zTq[:, j * 128:(j + 1) * 128],
                                     rhs=V[:, h * CH:(h + 1) * CH],
                                     start=True, stop=True)
                    outj = pool.tile([128, CH], BF, tag=f"outsb{j}{h}",
                                     name=f"outsb{j}{h}")
                    if h == 0:
                        nc.scalar.activation(outj[:], psO[:], AF.Identity,
                                             bias=qsb[:, j:j + 1], scale=1.0)
                    else:
                        nc.vector.tensor_scalar(outj[:], psO[:],
                                                qsb[:, j:j + 1], None,
                                                ALU.add)
                    out_eng[j].dma_start(
                        out[j * 128:(j + 1) * 128, h * CH:(h + 1) * CH],
                        outj[:])

    nc.compile()
    return nc


_NC_CACHE = None


def _get_program():
    global _NC_CACHE
    if _NC_CACHE is None:
        _NC_CACHE = build_program()
    return _NC_CACHE


def make_in_maps(z, y):
    z = np.ascontiguousarray(np.asarray(z, dtype=np.float32))
    yf = np.asarray(y).astype(np.float32)
    zt = z.reshape(NT, 128, D).transpose(1, 0, 2)          # (128, 16, 64)
    zm_np = np.ones((128, NT, D + 1), np.float16)
    zm_np[:, :, 0:D] = zt.astype(np.float16)
    yc_np = np.ascontiguousarray(yf.reshape(NT, 128).T.astype(np.float32))
    cv_np = np.arange(C, dtype=np.float16)
    in_maps = []
    for k in range(NCORES):
        czt_np = np.zeros((128, C + BL), np.float16)
        czt_np[:, 0:C] = cv_np
        czt_np[0:D, C:C + BL] = z[k * BL:(k + 1) * BL].T.astype(np.float16)
        czt_np[D, C:C + BL] = 1.0
        in_maps.append({
            "zm_in": zm_np,
            "czt_in": czt_np,
            "yc_in": yc_np,
        })
    return in_maps


def run(z, y, trace=False, **kwargs):
    nc = _get_program()
    res = run_bass_kernel_spmd(nc, make_in_maps(z, y), list(range(NCORES)),
                               trace=trace, **kwargs)
    full = np.concatenate(
        [np.asarray(res.results[k]["out_loc"]).astype(np.float32)
         for k in range(NCORES)], axis=0)
    return full, res


def kernel(z, y):
    full, _ = run(z, y, trace=False)
    return full


if __name__ == "__main__":
    rng = np.random.default_rng(0)
    z = rng.standard_normal((B, D), dtype=np.float32)
    y = rng.integers(0, C, size=(B,)).astype(np.int64)
    out = kernel(z, y)
    print("out", out.shape, out.dtype, out[0, :4])
